# revision 5
# baseline (speedup 1.0000x reference)
"""Trainium2 Bass kernel for nn_DivergenceRN (gnn_message_passing).

Reference computes, per batch b:
    Z_XX[b,i,:] = max_j relu(X[b,j]@W1a_xx + X[b,i]@W1c_xx + b1_xx) @ W_xx2
    Z_YX[b,i,:] = max_j relu(Y[b,j]@W1a_yx + X[b,i]@W1c_yx + b1_yx) @ W_yx2
    Z = sum_i (Z_XX - Z_YX);  out = relu(cat(Z,Z)@Wd1+bd1)@Wd2+bd2
(The YY / XY branches in the reference are dead code - output-independent.)

v3: the j-only term PA[b] = blockdiag(W1a)^T @ [X^T;Y^T] is computed ONCE per
batch (4 setup matmuls) and parked in SBUF as fp16; the i-only bias
pc_i = C[b,i]@W1[D:]+b1 is computed on the HOST and shipped as [128, BI] f32.
Per i the engines then do the minimum possible:
  relu: rp_i = Relu(PA[b] + pc_i) - split between ACT (bias-AP activation,
        ~614 ns) and DVE (tensor_scalar add+max0 at 2x fp16, ~383 ns)
  PE  : one [128x128] @ [128,384] W2 matmul (fp16, fixed weights)
  DVE : batched max-reduce over j (G=4 i's per op, the 1x-rate bottleneck)
Partitions: 64 h-channels x {xx, yx} = 128.  Sharding: i in [0,384) split
across 8 cores.  Host does the cross-core sum + b2/decoder folding (tiny).
"""

import numpy as np

import concourse.bacc as bacc
import concourse.mybir as mybir
import concourse.tile as tile
from concourse.bass_utils import run_bass_kernel_spmd

B, N, M, D, H = 4, 384, 384, 64, 64
NCORES = 8
NI = N // NCORES          # i-rows per core per batch
BI = B * NI               # i-rows per core total
P = 2 * H                 # 128 partitions: h x {xx, yx}
BLOB16_W = B * N + 2 * P  # xyt + w1ad + w2bd columns (fp16 blob)
DVE_EVERY = 5             # 1 of every 5 relus goes to the DVE, rest to ACT

F32 = mybir.dt.float32
FP16 = mybir.dt.float16
AX = mybir.AxisListType
ALU = mybir.AluOpType
ACTF = mybir.ActivationFunctionType


def build_nc():
    nc = bacc.Bacc("TRN2", target_bir_lowering=False)

    # fp16 blob: [X^T|Y^T] per b (interleaved on partitions), W1ad, W2bd.
    blob16 = nc.dram_tensor("blob16", [P, BLOB16_W], FP16, kind="ExternalInput")
    # Host-computed per-i bias pc (+b1 folded in), f32.
    pcin = nc.dram_tensor("pcin", [P, BI], F32, kind="ExternalInput")
    out = nc.dram_tensor("out", [P, B], F32, kind="ExternalOutput")

    with tile.TileContext(nc) as tc:
        with (
            tc.tile_pool(name="singles", bufs=1) as singles,
            tc.tile_pool(name="rp", bufs=14) as rp_pool,
            tc.tile_pool(name="hps", bufs=2, space="PSUM") as h_pool,
        ):
            blob_s = singles.tile([P, BLOB16_W], FP16)
            pc_s = singles.tile([P, BI], F32)
            pa_s = singles.tile([P, B, N], FP16)
            strip = singles.tile([P, B, NI], F32)
            acc = singles.tile([P, B], F32)

            # Load the Relu table before any data lands.
            warm = singles.tile([P, 1], F32)
            nc.vector.memset(warm, 0.0)
            nc.scalar.activation(out=warm, in_=warm, func=ACTF.Relu, scale=1.0)

            nc.sync.dma_start(out=blob_s[:, :], in_=blob16[:, :])
            nc.sync.dma_start(out=pc_s, in_=pcin[:, :])
            o = 0
            xyt_s = blob_s[:, o : o + B * N].rearrange("p (b n) -> p b n", b=B)
            o += B * N
            w1ad_s = blob_s[:, o : o + P]
            o += P
            w2_s = blob_s[:, o : o + P]
            o += P
            assert o == BLOB16_W

            # Setup: PA[b] = W1ad^T @ XYT[b] for all b into one 4-bank PSUM
            # tile, then one batched copy to SBUF fp16.
            setup_ps = h_pool.tile([P, B, 512], F32, tag="h")
            for b in range(B):
                nc.tensor.matmul(
                    setup_ps[:, b, 0:N],
                    lhsT=w1ad_s,
                    rhs=xyt_s[:, b, :],
                    start=True, stop=True,
                )
                # per-b fp16 copy on the DVE so b=0 relus start early and
                # the ACT engine stays free for the main loop
                nc.vector.tensor_copy(pa_s[:, b, :], setup_ps[:, b, 0:N])

            G = 4
            for b in range(B):
                for ig in range(NI // G):
                    h_ps = h_pool.tile([P, G, 512], F32, tag="h")
                    for g in range(G):
                        il = ig * G + g
                        bi = b * NI + il
                        rp = rp_pool.tile([P, N], FP16)
                        if il % DVE_EVERY == DVE_EVERY - 1:
                            # DVE: rp = max(PA + pc_i, 0) at 2x fp16
                            nc.vector.tensor_scalar(
                                out=rp, in0=pa_s[:, b, :],
                                scalar1=pc_s[:, bi : bi + 1], scalar2=0.0,
                                op0=ALU.add, op1=ALU.max,
                            )
                        else:
                            # ACT: rp = Relu(PA + pc_i)
                            nc.scalar.activation(
                                out=rp, in_=pa_s[:, b, :],
                                func=ACTF.Relu,
                                bias=pc_s[:, bi : bi + 1], scale=1.0,
                            )
                        nc.tensor.matmul(
                            h_ps[:, g, 0:N],
                            lhsT=w2_s,
                            rhs=rp,
                            start=True, stop=True,
                        )
                    nc.vector.tensor_reduce(
                        out=strip[:, b, ig * G : ig * G + G],
                        in_=h_ps[:, :, 0:N],
                        axis=AX.X,
                        op=ALU.max,
                    )

            nc.vector.tensor_reduce(
                out=acc[:, :], in_=strip[:, :, :], axis=AX.X, op=ALU.add
            )
            nc.sync.dma_start(out=out[:, :], in_=acc[:, :])

    nc.compile()
    return nc


def _prep_inputs(X, Y, W_xx1, b_xx1, W_yx1, b_yx1, W_xx2, W_yx2):
    """Host-side prep of the shared (non-i-sharded) device inputs."""
    f16 = np.float16
    blob = np.zeros((P, BLOB16_W), f16)
    o = 0
    xyt = np.concatenate(
        [X.transpose(0, 2, 1), Y.transpose(0, 2, 1)], axis=1
    )  # [B, 128, N]
    blob[:, o : o + B * N] = xyt.transpose(1, 0, 2).reshape(P, B * N).astype(f16)
    o += B * N
    w1ad = np.zeros((P, P), f16)
    w1ad[:D, :H] = W_xx1[:D].astype(f16)
    w1ad[D:, H:] = W_yx1[:D].astype(f16)
    blob[:, o : o + P] = w1ad
    o += P
    w2 = np.zeros((P, P), f16)
    w2[:H, :H] = W_xx2.astype(f16)
    w2[H:, H:] = W_yx2.astype(f16)
    blob[:, o : o + P] = w2
    o += P
    assert o == BLOB16_W
    # Host-computed per-i bias, full [128, B, N] then sliced per core.
    pc_xx = (X @ W_xx1[D:] + b_xx1).transpose(2, 0, 1)  # [64, B, N]
    pc_yx = (X @ W_yx1[D:] + b_yx1).transpose(2, 0, 1)  # [64, B, N]
    pc = np.concatenate([pc_xx, pc_yx], axis=0).astype(np.float32)  # [128,B,N]
    return blob, pc


def kernel(
    X, Y,
    W_xx1, b_xx1, W_xx2, b_xx2,
    W_xy1, b_xy1, W_xy2, b_xy2,
    W_yx1, b_yx1, W_yx2, b_yx2,
    W_yy1, b_yy1, W_yy2, b_yy2,
    Wd1, bd1, Wd2, bd2,
    _trace=False, _tmpdir=None,
):
    f = np.float32
    X = np.asarray(X, f)
    Y = np.asarray(Y, f)
    blob, pc = _prep_inputs(X, Y, W_xx1, b_xx1, W_yx1, b_yx1, W_xx2, W_yx2)

    in_maps = []
    for c in range(NCORES):
        pcc = pc[:, :, c * NI : (c + 1) * NI].reshape(P, BI)
        in_maps.append({"blob16": blob, "pcin": np.ascontiguousarray(pcc)})

    nc = build_nc()
    res = run_bass_kernel_spmd(
        nc,
        in_maps,
        core_ids=list(range(NCORES)),
        trace=_trace,
        tmpdir=_tmpdir,
    )
    acc = np.zeros((P, B), np.float64)
    for r in res.results:
        acc += r["out"].astype(np.float64)
    acc = acc.astype(f)

    # acc[k, b] = sum_i max_j (relu_pre @ W2)[k]  for xx (k<64) / yx (k>=64)
    Zdiff = (acc[:H] - acc[H:]).T + N * (b_xx2 - b_yx2)[None, :]  # [B, H]
    z = np.concatenate([Zdiff, Zdiff], axis=1).astype(f)  # [B, 2H]
    h = np.maximum(z @ Wd1 + bd1, 0.0).astype(f)
    outv = (h @ Wd2 + bd2).astype(f)
    if _trace:
        return outv, res
    return outv


# revision 6
# speedup vs baseline: 1.1141x; 1.1141x over previous
"""Trainium2 Bass kernel for nn_DivergenceRN (gnn_message_passing).

Reference computes, per batch b:
    Z_XX[b,i,:] = max_j relu(X[b,j]@W1a_xx + X[b,i]@W1c_xx + b1_xx) @ W_xx2
    Z_YX[b,i,:] = max_j relu(Y[b,j]@W1a_yx + X[b,i]@W1c_yx + b1_yx) @ W_yx2
    Z = sum_i (Z_XX - Z_YX);  out = relu(cat(Z,Z)@Wd1+bd1)@Wd2+bd2
(The YY / XY branches in the reference are dead code - output-independent.)

v3: the j-only term PA[b] = blockdiag(W1a)^T @ [X^T;Y^T] is computed ONCE per
batch (4 setup matmuls) and parked in SBUF as fp16; the i-only bias
pc_i = C[b,i]@W1[D:]+b1 is computed on the HOST and shipped as [128, BI] f32.
Per i the engines then do the minimum possible:
  relu: rp_i = Relu(PA[b] + pc_i) - split between ACT (bias-AP activation,
        ~614 ns) and DVE (tensor_scalar add+max0 at 2x fp16, ~383 ns)
  PE  : one [128x128] @ [128,384] W2 matmul (fp16, fixed weights)
  DVE : batched max-reduce over j (G=4 i's per op, the 1x-rate bottleneck)
Partitions: 64 h-channels x {xx, yx} = 128.  Sharding: i in [0,384) split
across 8 cores.  Host does the cross-core sum + b2/decoder folding (tiny).
"""

import numpy as np

import concourse.bacc as bacc
import concourse.mybir as mybir
import concourse.tile as tile
from concourse.bass_utils import run_bass_kernel_spmd

B, N, M, D, H = 4, 384, 384, 64, 64
NCORES = 8
NI = N // NCORES          # i-rows per core per batch
BI = B * NI               # i-rows per core total
P = 2 * H                 # 128 partitions: h x {xx, yx}
BLOB16_W = B * N + 2 * P  # xyt + w1ad + w2bd columns (fp16 blob)
DVE_EVERY = 6             # 1 of every 6 relus goes to the DVE, rest to ACT

F32 = mybir.dt.float32
FP16 = mybir.dt.float16
AX = mybir.AxisListType
ALU = mybir.AluOpType
ACTF = mybir.ActivationFunctionType


def build_nc():
    nc = bacc.Bacc("TRN2", target_bir_lowering=False)

    # fp16 blob: [X^T|Y^T] per b (interleaved on partitions), W1ad, W2bd.
    blob16 = nc.dram_tensor("blob16", [P, BLOB16_W], FP16, kind="ExternalInput")
    # Host-computed per-i bias pc (+b1 folded in), f32.
    pcin = nc.dram_tensor("pcin", [P, BI], F32, kind="ExternalInput")
    out = nc.dram_tensor("out", [P, B], F32, kind="ExternalOutput")

    with tile.TileContext(nc) as tc:
        with (
            tc.tile_pool(name="singles", bufs=1) as singles,
            tc.tile_pool(name="rp", bufs=14) as rp_pool,
            tc.tile_pool(name="hps", bufs=2, space="PSUM") as h_pool,
        ):
            blob_s = singles.tile([P, BLOB16_W], FP16)
            pc_s = singles.tile([P, BI], F32)
            pa_s = singles.tile([P, B, N], FP16)
            strip = singles.tile([P, B, NI], F32)
            acc = singles.tile([P, B], F32)

            # Load the Relu table before any data lands.
            warm = singles.tile([P, 1], F32)
            nc.vector.memset(warm, 0.0)
            nc.scalar.activation(out=warm, in_=warm, func=ACTF.Relu, scale=1.0)

            nc.sync.dma_start(out=blob_s[:, :], in_=blob16[:, :])
            nc.sync.dma_start(out=pc_s, in_=pcin[:, :])
            o = 0
            xyt_s = blob_s[:, o : o + B * N].rearrange("p (b n) -> p b n", b=B)
            o += B * N
            w1ad_s = blob_s[:, o : o + P]
            o += P
            w2_s = blob_s[:, o : o + P]
            o += P
            assert o == BLOB16_W

            # Setup: PA[b] = W1ad^T @ XYT[b] for all b into one 4-bank PSUM
            # tile, then one batched copy to SBUF fp16.
            setup_ps = h_pool.tile([P, B, 512], F32, tag="h")
            for b in range(B):
                nc.tensor.matmul(
                    setup_ps[:, b, 0:N],
                    lhsT=w1ad_s,
                    rhs=xyt_s[:, b, :],
                    start=True, stop=True,
                )
                # per-b fp16 copy on the DVE so b=0 relus start early and
                # the ACT engine stays free for the main loop
                nc.vector.tensor_copy(pa_s[:, b, :], setup_ps[:, b, 0:N])

            G = 4
            for b in range(B):
                for ig in range(NI // G):
                    h_ps = h_pool.tile([P, G, 512], F32, tag="h")
                    for g in range(G):
                        il = ig * G + g
                        bi = b * NI + il
                        rp = rp_pool.tile([P, N], FP16)
                        if il % DVE_EVERY == DVE_EVERY - 1:
                            # DVE: rp = max(PA + pc_i, 0) at 2x fp16
                            nc.vector.tensor_scalar(
                                out=rp, in0=pa_s[:, b, :],
                                scalar1=pc_s[:, bi : bi + 1], scalar2=0.0,
                                op0=ALU.add, op1=ALU.max,
                            )
                        else:
                            # ACT: rp = Relu(PA + pc_i)
                            nc.scalar.activation(
                                out=rp, in_=pa_s[:, b, :],
                                func=ACTF.Relu,
                                bias=pc_s[:, bi : bi + 1], scale=1.0,
                            )
                        nc.tensor.matmul(
                            h_ps[:, g, 0:N],
                            lhsT=w2_s,
                            rhs=rp,
                            start=True, stop=True,
                        )
                    nc.vector.tensor_reduce(
                        out=strip[:, b, ig * G : ig * G + G],
                        in_=h_ps[:, :, 0:N],
                        axis=AX.X,
                        op=ALU.max,
                    )

            nc.vector.tensor_reduce(
                out=acc[:, :], in_=strip[:, :, :], axis=AX.X, op=ALU.add
            )
            nc.sync.dma_start(out=out[:, :], in_=acc[:, :])

    nc.compile()
    return nc


def _prep_inputs(X, Y, W_xx1, b_xx1, W_yx1, b_yx1, W_xx2, W_yx2):
    """Host-side prep of the shared (non-i-sharded) device inputs."""
    f16 = np.float16
    blob = np.zeros((P, BLOB16_W), f16)
    o = 0
    xyt = np.concatenate(
        [X.transpose(0, 2, 1), Y.transpose(0, 2, 1)], axis=1
    )  # [B, 128, N]
    blob[:, o : o + B * N] = xyt.transpose(1, 0, 2).reshape(P, B * N).astype(f16)
    o += B * N
    w1ad = np.zeros((P, P), f16)
    w1ad[:D, :H] = W_xx1[:D].astype(f16)
    w1ad[D:, H:] = W_yx1[:D].astype(f16)
    blob[:, o : o + P] = w1ad
    o += P
    w2 = np.zeros((P, P), f16)
    w2[:H, :H] = W_xx2.astype(f16)
    w2[H:, H:] = W_yx2.astype(f16)
    blob[:, o : o + P] = w2
    o += P
    assert o == BLOB16_W
    # Host-computed per-i bias, full [128, B, N] then sliced per core.
    pc_xx = (X @ W_xx1[D:] + b_xx1).transpose(2, 0, 1)  # [64, B, N]
    pc_yx = (X @ W_yx1[D:] + b_yx1).transpose(2, 0, 1)  # [64, B, N]
    pc = np.concatenate([pc_xx, pc_yx], axis=0).astype(np.float32)  # [128,B,N]
    return blob, pc


def kernel(
    X, Y,
    W_xx1, b_xx1, W_xx2, b_xx2,
    W_xy1, b_xy1, W_xy2, b_xy2,
    W_yx1, b_yx1, W_yx2, b_yx2,
    W_yy1, b_yy1, W_yy2, b_yy2,
    Wd1, bd1, Wd2, bd2,
    _trace=False, _tmpdir=None,
):
    f = np.float32
    X = np.asarray(X, f)
    Y = np.asarray(Y, f)
    blob, pc = _prep_inputs(X, Y, W_xx1, b_xx1, W_yx1, b_yx1, W_xx2, W_yx2)

    in_maps = []
    for c in range(NCORES):
        pcc = pc[:, :, c * NI : (c + 1) * NI].reshape(P, BI)
        in_maps.append({"blob16": blob, "pcin": np.ascontiguousarray(pcc)})

    nc = build_nc()
    res = run_bass_kernel_spmd(
        nc,
        in_maps,
        core_ids=list(range(NCORES)),
        trace=_trace,
        tmpdir=_tmpdir,
    )
    acc = np.zeros((P, B), np.float64)
    for r in res.results:
        acc += r["out"].astype(np.float64)
    acc = acc.astype(f)

    # acc[k, b] = sum_i max_j (relu_pre @ W2)[k]  for xx (k<64) / yx (k>=64)
    Zdiff = (acc[:H] - acc[H:]).T + N * (b_xx2 - b_yx2)[None, :]  # [B, H]
    z = np.concatenate([Zdiff, Zdiff], axis=1).astype(f)  # [B, 2H]
    h = np.maximum(z @ Wd1 + bd1, 0.0).astype(f)
    outv = (h @ Wd2 + bd2).astype(f)
    if _trace:
        return outv, res
    return outv
